# revision 1
# baseline (speedup 1.0000x reference)
"""Self-contained Trainium2 Bass kernel: mean symmetric point-to-closest-point
(Chamfer) distance between batches of 2048-point 2D clouds.

Problem: outputs/targets (32, 4096) fp32 -> point clouds (32, 2048, 2);
result = mean_b 0.5*(mean_i min_j d_ij + mean_j min_i d_ij), a fp32 scalar.

Sharding: data parallel over the batch dim — core c computes batches
4c..4c+3; each core returns partial sums of sqrt(min d^2) in res[128, 2];
the host sums and scales (an all-reduce-mean equivalent done host-side
since the output is a scalar).

Device algorithm per core (4 batches):
  * D2[i,j] = ||u_i||^2 + ||v_j||^2 - 2 u_i.v_j is computed on the
    TensorEngine as a K=10 matmul with fp16 hi/lo-split operands
    (fp32-grade accuracy at full 1 cycle/row PE rate), 512 cols per
    PSUM bank, 4-way double-buffered across the 8 banks.
  * ScalarEngine evacuates each PSUM tile to SBUF fp16 with a fused
    Relu clamp, enabling DVE 2x packed-fp16 mode.
  * Row mins (u->v): per-i-tile TT-min folds collected into a per-batch
    buffer, finished by an in-place 2x fold tree + one 1x reduce.
    Col mins (v->u): running TT-min accumulator, finalized with PE
    transposes + a free-dim min reduce straight from PSUM.
  * sqrt + sums via ScalarEngine Sqrt activation with fused sum
    accumulation; [128, 2] partials DMA'd out per core.

Notes from HW bring-up: DVE ops with accum_out (tensor_tensor_reduce,
tensor_scalar+accum) crash or fail this environment's compiler/runtime,
and GPSIMD tensor_tensor fails walrus codegen - hence the fold-based
reductions. ScalarE activation accum (sum) works.
"""
from contextlib import ExitStack

import numpy as np

import concourse.bacc as bacc
import concourse.tile as tile
from concourse import mybir
from concourse.bass_utils import run_bass_kernel_spmd

F16 = mybir.dt.float16
F32 = mybir.dt.float32
MIN = mybir.AluOpType.min

N_CORES = 8
NB = 4          # batches per core
NPT = 2048      # points per cloud
NT = 16         # 128-point i-tiles per batch


def _emit_body(nc, out_d, tgt_d, ident_d, res_d, pools, ablate=()):
    sing, work, pp = pools

    ident = sing.tile([128, 128], F16, name="ident")
    nc.sync.dma_start(out=ident, in_=ident_d[:, :])

    # ---- load raw coords as [128, 16] per batch: i = p*16+g ----
    raw = {}
    for nm, dram, lo in (("ux", out_d, 0), ("uy", out_d, NPT),
                         ("vx", tgt_d, 0), ("vy", tgt_d, NPT)):
        t = sing.tile([128, NB * 16], F32, name=f"raw_{nm}")
        for b in range(NB):
            eng = nc.sync if (b % 2 == 0) else nc.gpsimd
            eng.dma_start(
                out=t[:, b * 16:(b + 1) * 16],
                in_=dram[b:b + 1, lo:lo + NPT].rearrange("o (p g) -> (o p) g", g=16),
            )
        raw[nm] = t

    # ---- fp16 hi/lo splits at [128, 64] granularity ----
    # pack_u vectors: 0 nu_hi, 1 nu_lo, 2 uxhi, 3 uxlo, 4 uyhi, 5 uylo
    # pack_v vectors: 0 nv_hi, 1 nv_lo, 2 -2vxhi, 3 -2vxlo, 4 -2vyhi, 5 -2vylo
    pack_u = sing.tile([128, NB, 6, 16], F16, name="pack_u")
    pack_v = sing.tile([128, NB, 6, 16], F16, name="pack_v")

    for side, (cx, cy), pack in (("u", ("ux", "uy"), pack_u),
                                 ("v", ("vx", "vy"), pack_v)):
        x, y = raw[cx], raw[cy]
        sq = work.tile([128, NB * 16], F32, name=f"sq_{side}", tag="pre32")
        nrm = work.tile([128, NB * 16], F32, name=f"nrm_{side}", tag="pre32b")
        nc.vector.tensor_mul(sq, x, x)
        nc.vector.tensor_mul(nrm, y, y)
        nc.vector.tensor_tensor(nrm, sq, nrm, op=mybir.AluOpType.add)
        nc.vector.tensor_copy(pack[:, :, 0, :], nrm)
        nc.vector.tensor_sub(pack[:, :, 1, :], nrm, pack[:, :, 0, :])
        if side == "u":
            nc.vector.tensor_copy(pack[:, :, 2, :], x)
            nc.vector.tensor_sub(pack[:, :, 3, :], x, pack[:, :, 2, :])
            nc.vector.tensor_copy(pack[:, :, 4, :], y)
            nc.vector.tensor_sub(pack[:, :, 5, :], y, pack[:, :, 4, :])
        else:
            xhi = work.tile([128, NB * 16], F16, name="xhi", tag="pre16")
            xlo = work.tile([128, NB * 16], F16, name="xlo", tag="pre16b")
            nc.vector.tensor_copy(xhi, x)
            nc.vector.tensor_sub(xlo, x, xhi)
            nc.vector.tensor_scalar_mul(pack[:, :, 2, :], xhi, -2.0)
            nc.vector.tensor_scalar_mul(pack[:, :, 3, :], xlo, -2.0)
            yhi = work.tile([128, NB * 16], F16, name="yhi", tag="pre16")
            ylo = work.tile([128, NB * 16], F16, name="ylo", tag="pre16b")
            nc.vector.tensor_copy(yhi, y)
            nc.vector.tensor_sub(ylo, y, yhi)
            nc.vector.tensor_scalar_mul(pack[:, :, 4, :], yhi, -2.0)
            nc.vector.tensor_scalar_mul(pack[:, :, 5, :], ylo, -2.0)

    # ---- per-batch transpose + assembly of W_b, M_b [10, 2048] fp16 ----
    # W rows: [nu_hi, nu_lo, 1, 1, uxhi, uxhi, uxlo, uyhi, uyhi, uylo]
    # M rows: [1, 1, nv_hi, nv_lo, -2vxhi, -2vxlo, -2vxhi, -2vyhi, -2vylo, -2vyhi]
    # D2 column order: c = m*128 + q  <->  i = q*16 + m (consistent bijection)
    Ws, Ms = [], []
    W_ROWS = [0, 1, None, None, 2, 2, 3, 4, 4, 5]   # None -> ones
    M_ROWS = [None, None, 0, 1, 2, 3, 2, 4, 5, 4]
    ones_sb = sing.tile([2, NPT], F16, name="ones_sb")
    nc.vector.memset(ones_sb, 1.0)
    for b in range(NB):
        for pack, rows, out_list, nm in ((pack_u, W_ROWS, Ws, "W"),
                                         (pack_v, M_ROWS, Ms, "M")):
            tp = pp.tile([96, 128], F16, name=f"tp_{nm}{b}", tag="ps", bufs=2)
            nc.tensor.transpose(tp, pack[:, b, :, :].rearrange("p a g -> p (a g)"), ident)
            tsb = work.tile([96, 128], F16, name=f"tsb_{nm}{b}", tag="tsb")
            nc.scalar.copy(tsb, tp)
            buf = sing.tile([10, NPT], F16, name=f"{nm}{b}")
            ones_done = False
            qi = 0
            for r, v in enumerate(rows):
                if v is None:
                    if not ones_done:
                        nc.sync.dma_start(out=buf[r:r + 2, :], in_=ones_sb[:, :])
                        ones_done = True
                else:
                    eng = nc.sync if (qi % 2 == 0) else nc.gpsimd
                    qi += 1
                    eng.dma_start(
                        out=buf[r:r + 1, :].rearrange("o (m q) -> o m q", m=16),
                        in_=tsb[v * 16:(v + 1) * 16, :],
                    )
            out_list.append(buf)

    # ---- main loop ----
    rowmins = sing.tile([128, NB * NT], F32, name="rowmins")
    colmins = sing.tile([128, NB * NT], F32, name="colmins")
    for b in range(NB):
        W, M = Ws[b], Ms[b]
        colacc = work.tile([128, NPT], F16, name=f"colacc{b}", tag="colacc")
        s2all = work.tile([128, NT, NPT // 4], F16, name=f"s2all{b}",
                          tag="s2all", bufs=2)
        for t in range(NT):
            c = None if "act" in ablate else work.tile(
                [128, NPT], F16, name=f"c{b}_{t}", tag="c")
            ps = pp.tile([128, NPT], F32, name=f"ps{b}_{t}", tag="ps", bufs=2)
            for n in range(4):
                nc.tensor.matmul(
                    ps[:, 512 * n:512 * (n + 1)],
                    W[:, 128 * t:128 * (t + 1)],
                    M[:, 512 * n:512 * (n + 1)],
                    start=True, stop=True,
                )
            if c is not None:
                nc.scalar.activation(c, ps,
                                     mybir.ActivationFunctionType.Relu)
            if "act" in ablate:
                continue
            if "rowmin" not in ablate:
                if t % 2 == 0:
                    s1p = work.tile([128, 2, NPT // 2], F16, name=f"s1p{b}_{t}",
                                    tag="s1p", bufs=3)
                nc.vector.tensor_tensor(
                    s1p[:, t % 2, :], c[:, :NPT // 2], c[:, NPT // 2:], op=MIN)
                if t % 2 == 1:
                    nc.vector.tensor_tensor(
                        s2all[:, t - 1:t + 1, :], s1p[:, :, :NPT // 4],
                        s1p[:, :, NPT // 4:], op=MIN)
            if "colmin" in ablate:
                continue
            if t == 0:
                nc.vector.tensor_copy(colacc, c)
            else:
                nc.vector.tensor_tensor(colacc, c, colacc, op=MIN)
        # ---- batched row-min reduce: in-place 2x tree folds, then reduce ----
        if "rowmin" not in ablate and "act" not in ablate:
            w = NPT // 4
            while w > 32:
                nc.vector.tensor_tensor(
                    s2all[:, :, :w // 2], s2all[:, :, :w // 2],
                    s2all[:, :, w // 2:w], op=MIN)
                w //= 2
            nc.vector.tensor_reduce(
                out=rowmins[:, b * NT:(b + 1) * NT], in_=s2all[:, :, :w],
                axis=mybir.AxisListType.X, op=MIN,
            )
        # ---- col-min finalize: PE transposes + reduce straight from PSUM ----
        if "colmin" in ablate or "act" in ablate:
            continue
        pst = pp.tile([128, NPT], F16, name=f"pst{b}", tag="ps", bufs=2)
        for k in range(NT):
            nc.tensor.transpose(
                pst[:, 128 * k:128 * (k + 1)],
                colacc[:, 128 * k:128 * (k + 1)],
                ident,
            )
        nc.vector.tensor_reduce(
            out=colmins[:, b * NT:(b + 1) * NT],
            in_=pst.rearrange("p (k q) -> p k q", k=NT),
            axis=mybir.AxisListType.X, op=MIN,
        )

    # ---- epilogue: clamp, sqrt, fused sum ----
    res_sb = sing.tile([128, 2], F32, name="res_sb")
    junk = work.tile([128, NB * NT], F32, name="junk", tag="junk")
    nc.scalar.activation(junk, rowmins, mybir.ActivationFunctionType.Sqrt,
                         accum_out=res_sb[:, 0:1])
    nc.scalar.activation(junk, colmins, mybir.ActivationFunctionType.Sqrt,
                         accum_out=res_sb[:, 1:2])
    nc.sync.dma_start(out=res_d[:, :], in_=res_sb)


def build_kernel(reps: int = 1, ablate=()):
    nc = bacc.Bacc("TRN2", target_bir_lowering=False, debug=False)
    out_d = nc.dram_tensor("outputs", [NB, 2 * NPT], F32, kind="ExternalInput")
    tgt_d = nc.dram_tensor("targets", [NB, 2 * NPT], F32, kind="ExternalInput")
    ident_d = nc.dram_tensor("ident", [128, 128], F16, kind="ExternalInput")
    res_d = nc.dram_tensor("res", [128, 2], F32, kind="ExternalOutput")
    with tile.TileContext(nc) as tc:
        with ExitStack() as ctx:
            sing = ctx.enter_context(tc.tile_pool(name="sing", bufs=1))
            work = ctx.enter_context(tc.tile_pool(name="work", bufs=6))
            pp = ctx.enter_context(tc.tile_pool(name="pp", bufs=4, space="PSUM"))
            pools = (sing, work, pp)
            if reps == 1:
                _emit_body(nc, out_d, tgt_d, ident_d, res_d, pools, ablate)
            else:
                with tc.For_i(0, reps, 1):
                    _emit_body(nc, out_d, tgt_d, ident_d, res_d, pools, ablate)
    nc.compile()
    return nc


_NC_CACHE = {}


def _get_nc(reps: int = 1):
    if reps not in _NC_CACHE:
        _NC_CACHE[reps] = build_kernel(reps)
    return _NC_CACHE[reps]


def kernel(outputs: np.ndarray, targets: np.ndarray) -> np.ndarray:
    outputs = np.ascontiguousarray(outputs, dtype=np.float32)
    targets = np.ascontiguousarray(targets, dtype=np.float32)
    ident = np.eye(128, dtype=np.float16)
    nc = _get_nc(1)
    in_maps = [
        {
            "outputs": outputs[c * NB:(c + 1) * NB],
            "targets": targets[c * NB:(c + 1) * NB],
            "ident": ident,
        }
        for c in range(N_CORES)
    ]
    res = run_bass_kernel_spmd(nc, in_maps, core_ids=list(range(N_CORES)))
    s = np.float64(0.0)
    for r in res.results:
        s += r["res"].astype(np.float64).sum()
    return np.float32(s * 0.5 / (NPT * NB * N_CORES))



# revision 8
# speedup vs baseline: 1.2034x; 1.2034x over previous
"""Self-contained Trainium2 Bass kernel: mean symmetric point-to-closest-point
(Chamfer) distance between batches of 2048-point 2D clouds.

Problem: outputs/targets (32, 4096) fp32 -> point clouds (32, 2048, 2);
result = mean_b 0.5*(mean_i min_j d_ij + mean_j min_i d_ij), a fp32 scalar.

Sharding: data parallel over the batch dim - core c computes batches
4c..4c+3; each core returns partial sums of sqrt(min d^2) in res[128, 2];
the host sums and scales (an all-reduce-mean equivalent done host-side
since the output is a scalar).

Device algorithm per core (4 batches):
  * D2[i,j] = ||u_i||^2 + ||v_j||^2 - 2 u_i.v_j is computed on the
    TensorEngine as a K=10 matmul with fp16 hi/lo-split operands
    (fp32-grade accuracy at full 1 cycle/row PE rate), 512 cols per
    PSUM bank, double-buffered across the 8 banks.
  * ScalarEngine evacuates each PSUM tile to SBUF fp16 with a fused
    Relu clamp, enabling DVE 2x packed-fp16 mode.
  * Row mins (u->v): paired-tile first-level folds into s1p2, then
    per-pair folds into a per-batch buffer finished by an in-place
    2x fold tree + one 1x reduce.
    Col mins (v->u): running TT-min accumulator (first op consumes the
    first two tiles directly - no init copy), finalized with PE
    transposes + 2x PSUM-fp16 folds + one small reduce.
  * sqrt + sums via ScalarEngine Sqrt activation with fused sum
    accumulation; [128, 2] partials DMA'd out per core.

Notes from HW bring-up: DVE ops with accum_out (tensor_tensor_reduce,
tensor_scalar+accum) crash this environment's runtime (verified: device
becomes unrecoverable), GPSIMD tensor_tensor fails walrus codegen
(verified: ISA check rejects TT on Pool), and DMA accum_op=min is
rejected by the compiler (verified) - hence all reductions stay on DVE.
InstMax (vector.max top-8) works but runs at 1x, slower than 2x fold
chains. ScalarE activation accum (sum) works.
"""
from contextlib import ExitStack

import numpy as np

import concourse.bacc as bacc
import concourse.tile as tile
from concourse import mybir
from concourse.bass_utils import run_bass_kernel_spmd

F16 = mybir.dt.float16
F32 = mybir.dt.float32
MIN = mybir.AluOpType.min

N_CORES = 8
NB = 4          # batches per core
NPT = 2048      # points per cloud
NT = 16         # 128-point i-tiles per batch


def _emit_consts(nc, ident_d, ones_d, sing):
    """Constant loads hoisted out of the rep loop."""
    ident = sing.tile([128, 128], F16, name="ident")
    nc.sync.dma_start(out=ident, in_=ident_d[:, :])
    ones_sb = sing.tile([2, NPT], F16, name="ones_sb")
    nc.sync.dma_start(out=ones_sb, in_=ones_d[:, :])
    return ident, ones_sb


def _emit_body(nc, out_d, tgt_d, res_d, ident, ones_sb, pools):
    work, pp = pools

    # ---- load raw coords as [128, 4, 16]: i = p*16+g, one DMA per stream ----
    raw = {}
    for nm, dram, lo in (("ux", out_d, 0), ("uy", out_d, NPT),
                         ("vx", tgt_d, 0), ("vy", tgt_d, NPT)):
        t = work.tile([128, NB, 16], F32, name=f"raw_{nm}", tag=f"raw_{nm}",
                      bufs=2)
        eng = nc.sync if nm in ("ux", "vx") else nc.gpsimd
        eng.dma_start(
            out=t,
            in_=dram[:, lo:lo + NPT].rearrange("b (p g) -> p b g", g=16),
        )
        raw[nm] = t

    # ---- fp16 hi/lo splits at [128, 64] granularity ----
    # Vector orders are chosen so W/M rows form contiguous blocks of the
    # transposed pack, collapsing the row-scatter DMAs into block DMAs.
    # pack_u vectors: 0 nu_hi, 1 nu_lo, 2 uxlo, 3 uylo, 4 uxhi, 5 uyhi
    # pack_v vectors: 0 -2vxhi, 1 -2vyhi, 2 nv_hi, 3 nv_lo, 4 -2vxlo, 5 -2vylo
    pack_u = work.tile([128, NB, 6, 16], F16, name="pack_u", tag="pku", bufs=2)
    pack_v = work.tile([128, NB, 6, 16], F16, name="pack_v", tag="pkv", bufs=2)

    for side, (cx, cy), pack in (("u", ("ux", "uy"), pack_u),
                                 ("v", ("vx", "vy"), pack_v)):
        x = raw[cx].rearrange("p b g -> p (b g)")
        y = raw[cy].rearrange("p b g -> p (b g)")
        sq = work.tile([128, NB * 16], F32, name=f"sq_{side}", tag="pre32")
        nrm = work.tile([128, NB * 16], F32, name=f"nrm_{side}", tag="pre32b")
        nc.vector.tensor_mul(sq, x, x)
        nc.vector.tensor_mul(nrm, y, y)
        nc.vector.tensor_tensor(nrm, sq, nrm, op=mybir.AluOpType.add)
        if side == "u":
            nc.vector.tensor_copy(pack[:, :, 0, :], nrm)
            nc.vector.tensor_sub(pack[:, :, 1, :], nrm, pack[:, :, 0, :])
            nc.vector.tensor_copy(pack[:, :, 4, :], x)
            nc.vector.tensor_sub(pack[:, :, 2, :], x, pack[:, :, 4, :])
            nc.vector.tensor_copy(pack[:, :, 5, :], y)
            nc.vector.tensor_sub(pack[:, :, 3, :], y, pack[:, :, 5, :])
        else:
            nc.vector.tensor_copy(pack[:, :, 2, :], nrm)
            nc.vector.tensor_sub(pack[:, :, 3, :], nrm, pack[:, :, 2, :])
            xhi = work.tile([128, NB * 16], F16, name="xhi", tag="pre16")
            xlo = work.tile([128, NB * 16], F16, name="xlo", tag="pre16b")
            nc.vector.tensor_copy(xhi, x)
            nc.vector.tensor_sub(xlo, x, xhi)
            nc.vector.tensor_scalar_mul(pack[:, :, 0, :], xhi, -2.0)
            nc.vector.tensor_scalar_mul(pack[:, :, 4, :], xlo, -2.0)
            yhi = work.tile([128, NB * 16], F16, name="yhi", tag="pre16")
            ylo = work.tile([128, NB * 16], F16, name="ylo", tag="pre16b")
            nc.vector.tensor_copy(yhi, y)
            nc.vector.tensor_sub(ylo, y, yhi)
            nc.vector.tensor_scalar_mul(pack[:, :, 1, :], yhi, -2.0)
            nc.vector.tensor_scalar_mul(pack[:, :, 5, :], ylo, -2.0)

    # ---- per-batch transpose + assembly of W_b, M_b [10, 2048] fp16 ----
    # K-term pairing (row k of W times row k of M):
    #   k0: nu_hi*1      k1: nu_lo*1     k2: uxlo*-2vxhi  k3: uylo*-2vyhi
    #   k4: uxhi*-2vxhi  k5: uyhi*-2vyhi k6: 1*nv_hi      k7: 1*nv_lo
    #   k8: uxhi*-2vxlo  k9: uyhi*-2vylo
    # W rows = [u0..u5, 1, 1, u4, u5]; M rows = [1, 1, v0, v1, v0, v1, v2..v5]
    # D2 column order: c = m*128 + q  <->  i = q*16 + m (consistent bijection)
    Ws, Ms = [], []
    # Batch 0's W/M gate the whole pipeline: spread its scatters over three
    # DMA queues (sync, gpsimd, scalar-HWDGE); later batches hide behind the
    # main loop on two queues.
    qs3 = [nc.sync, nc.gpsimd, nc.scalar]
    qi = 0
    for b in range(NB):
        # (dst_row_start, n_rows, src) with src None -> ones, int -> tsb row/16
        for pack, blocks, out_list, nm in (
                (pack_u, [(0, 6, 0), (8, 2, 4), (6, 2, None)], Ws, "W"),
                (pack_v, [(2, 2, 0), (4, 2, 0), (6, 4, 2), (0, 2, None)],
                 Ms, "M")):
            tp = pp.tile([96, 128], F16, name=f"tp_{nm}{b}", tag="ps", bufs=2)
            nc.tensor.transpose(tp, pack[:, b, :, :].rearrange("p a g -> p (a g)"), ident)
            tsb = work.tile([96, 128], F16, name=f"tsb_{nm}{b}", tag="tsb")
            nc.scalar.copy(tsb, tp)
            buf = work.tile([10, NPT], F16, name=f"{nm}{b}", tag=f"wm{nm}{b}",
                            bufs=2)
            for r0, nr, v in blocks:
                if b == 0:
                    eng = qs3[qi % 3]
                else:
                    eng = nc.sync if (qi % 2 == 0) else nc.gpsimd
                qi += 1
                if v is None:
                    eng.dma_start(out=buf[r0:r0 + nr, :], in_=ones_sb[:, :])
                else:
                    eng.dma_start(
                        out=buf[r0:r0 + nr, :].rearrange(
                            "r (m q) -> r m q", m=16),
                        in_=tsb[v * 16:(v + nr) * 16, :],
                    )
            out_list.append(buf)

    # ---- main loop ----
    # The rowmin tree + colmin finalize of batch b are deferred into batch
    # b+1's pipeline (emitted at its t==2 slot) so the batch boundary never
    # stalls DVE; sqrt results accumulate per batch into sqr/sqc.
    sqr = work.tile([128, NB * NT], F32, name="sqr", tag="sqr", bufs=2)
    sqc = work.tile([128, NB * NT], F32, name="sqc", tag="sqc", bufs=2)
    state = {}

    def emit_finalize(b):
        colacc, s2all = state[b]
        rowm = work.tile([128, NT], F32, name=f"rowm{b}", tag="rm", bufs=2)
        colm = work.tile([128, NT], F32, name=f"colm{b}", tag="cm", bufs=2)
        w = NPT // 4
        while w > 32:
            nc.vector.tensor_tensor(
                s2all[:, :, :w // 2], s2all[:, :, :w // 2],
                s2all[:, :, w // 2:w], op=MIN)
            w //= 2
        nc.vector.tensor_reduce(
            out=rowm, in_=s2all[:, :, :w], axis=mybir.AxisListType.X, op=MIN)
        # col-min finalize: PE transposes + reduce straight from PSUM.
        # (DVE may read only ONE non-scalar input from PSUM, so pairwise
        # folds of pst halves are illegal; a single 1x reduce is the best.)
        pst = pp.tile([128, NT, 128], F16, name=f"pst{b}", tag="ps", bufs=2)
        for k in range(NT):
            nc.tensor.transpose(
                pst[:, k, :],
                colacc[:, 128 * k:128 * (k + 1)],
                ident,
            )
        nc.vector.tensor_reduce(
            out=colm, in_=pst, axis=mybir.AxisListType.X, op=MIN)
        nc.scalar.activation(sqr[:, b * NT:(b + 1) * NT], rowm,
                             mybir.ActivationFunctionType.Sqrt)
        nc.scalar.activation(sqc[:, b * NT:(b + 1) * NT], colm,
                             mybir.ActivationFunctionType.Sqrt)

    for b in range(NB):
        W, M = Ws[b], Ms[b]
        colacc = work.tile([128, NPT], F16, name=f"colacc{b}", tag="colacc",
                           bufs=2)
        s2all = work.tile([128, NT, NPT // 4], F16, name=f"s2all{b}",
                          tag="s2all", bufs=2)
        state[b] = (colacc, s2all)
        for t in range(NT):
            if b > 0 and t == 2:
                emit_finalize(b - 1)
            if t % 2 == 0:
                cc = work.tile([128, 2, NPT], F16, name=f"cc{b}_{t}", tag="cc",
                               bufs=2)
            c = cc[:, t % 2, :]
            ps = pp.tile([128, NPT], F32, name=f"ps{b}_{t}", tag="ps", bufs=2)
            for n in range(4):
                nc.tensor.matmul(
                    ps[:, 512 * n:512 * (n + 1)],
                    W[:, 128 * t:128 * (t + 1)],
                    M[:, 512 * n:512 * (n + 1)],
                    start=True, stop=True,
                )
            nc.scalar.activation(c, ps, mybir.ActivationFunctionType.Relu)
            # ---- col-min accumulator ----
            if t == 1:
                nc.vector.tensor_tensor(colacc, cc[:, 0, :], cc[:, 1, :],
                                        op=MIN)
            elif t > 1:
                nc.vector.tensor_tensor(colacc, c, colacc, op=MIN)
            # ---- row-min: paired first-level fold + per-pair second fold ----
            if t % 2 == 1:
                s1p2 = work.tile([128, 2, NPT // 2], F16, name=f"s1p{b}_{t}",
                                 tag="s1p", bufs=2)
                nc.vector.tensor_tensor(
                    s1p2, cc[:, :, :NPT // 2], cc[:, :, NPT // 2:], op=MIN)
                nc.vector.tensor_tensor(
                    s2all[:, t - 1:t + 1, :], s1p2[:, :, :NPT // 4],
                    s1p2[:, :, NPT // 4:], op=MIN)
    emit_finalize(NB - 1)

    # ---- epilogue: fused sums of the per-batch sqrt tiles ----
    res_sb = work.tile([128, 2], F32, name="res_sb", tag="res", bufs=2)
    junk = work.tile([128, NB * NT], F32, name="junk", tag="junk")
    nc.scalar.activation(junk, sqr, mybir.ActivationFunctionType.Copy,
                         accum_out=res_sb[:, 0:1])
    nc.scalar.activation(junk, sqc, mybir.ActivationFunctionType.Copy,
                         accum_out=res_sb[:, 1:2])
    nc.sync.dma_start(out=res_d[:, :], in_=res_sb)


def build_kernel(reps: int = 1):
    nc = bacc.Bacc("TRN2", target_bir_lowering=False, debug=False)
    out_d = nc.dram_tensor("outputs", [NB, 2 * NPT], F32, kind="ExternalInput")
    tgt_d = nc.dram_tensor("targets", [NB, 2 * NPT], F32, kind="ExternalInput")
    ident_d = nc.dram_tensor("ident", [128, 128], F16, kind="ExternalInput")
    ones_d = nc.dram_tensor("ones", [2, NPT], F16, kind="ExternalInput")
    res_d = nc.dram_tensor("res", [128, 2], F32, kind="ExternalOutput")
    with tile.TileContext(nc) as tc:
        with ExitStack() as ctx:
            sing = ctx.enter_context(tc.tile_pool(name="sing", bufs=1))
            work = ctx.enter_context(tc.tile_pool(name="work", bufs=6))
            pp = ctx.enter_context(tc.tile_pool(name="pp", bufs=4, space="PSUM"))
            ident, ones_sb = _emit_consts(nc, ident_d, ones_d, sing)
            pools = (work, pp)
            if reps == 1:
                _emit_body(nc, out_d, tgt_d, res_d, ident, ones_sb, pools)
            else:
                with tc.For_i(0, reps, 1):
                    _emit_body(nc, out_d, tgt_d, res_d, ident, ones_sb, pools)
    nc.compile()
    return nc


_NC_CACHE = {}


def _get_nc(reps: int = 1):
    if reps not in _NC_CACHE:
        _NC_CACHE[reps] = build_kernel(reps)
    return _NC_CACHE[reps]


def kernel(outputs: np.ndarray, targets: np.ndarray) -> np.ndarray:
    outputs = np.ascontiguousarray(outputs, dtype=np.float32)
    targets = np.ascontiguousarray(targets, dtype=np.float32)
    ident = np.eye(128, dtype=np.float16)
    ones = np.ones((2, NPT), dtype=np.float16)
    nc = _get_nc(1)
    in_maps = [
        {
            "outputs": outputs[c * NB:(c + 1) * NB],
            "targets": targets[c * NB:(c + 1) * NB],
            "ident": ident,
            "ones": ones,
        }
        for c in range(N_CORES)
    ]
    res = run_bass_kernel_spmd(nc, in_maps, core_ids=list(range(N_CORES)))
    s = np.float64(0.0)
    for r in res.results:
        s += r["res"].astype(np.float64).sum()
    return np.float32(s * 0.5 / (NPT * NB * N_CORES))
